# revision 1
# baseline (speedup 1.0000x reference)
"""Trainium2 Bass kernel for CosineSimilarityWeightedAverage.

reference:
  input [B=4, C=4096, D=64] f32
  in_n = input / ||input||_row
  cos  = in_n @ in_n.T per batch            [B, C, C]
  attn = softmax(cos / 0.1, axis=-1)
  out  = (attn @ weight) * weight_global * input + (attn @ bias) * bias_global

Sharding: 8 cores = (batch b = core//2) x (query half h = core%2, 2048 rows).
Each core gets all 4096 keys of its batch and computes 2048 output rows.

Per-core kernel (all matmul operands fp16, accumulation fp32 in PSUM):
  - normalize keys by 10/||k|| (temperature folded in), queries by 1/||q||
  - transposed [64, C] layouts built with paired 2-byte DMA xbar transposes
  - stage 1: scoresT[k, q] = knT.T @ qnT  (K=64 contraction; even k-tiles at
    partitions 0-63, odd at 64-127 -> row-group concurrency on the PE array)
  - exp via one scalar-engine activation per psum batch (no max subtraction:
    logits in [-10, 10], exp in [4.5e-5, 2.2e4], fp32/fp16-safe)
  - stage 2: [W|bias] (128 cols) stationary per k-tile, accumulate over k
  - denominators: ones-vector matmul -> [1, 512] psum accumulators
  - finalize: reciprocal, partition-broadcast, normalize, PE transpose back
    to [q, d], combine avgW*(wg*x) + avgB*bg, DMA out.
"""

import numpy as np

B = 4
C = 4096
D = 64
TEMP = 0.1
NCORES = 8
CQ = C // 2          # queries per core
KT = C // 128        # 32 k-tiles
QT = CQ // 128       # 16 q-tiles

_CACHE = {}


def _build():
    import concourse.bass as bass
    import concourse.bacc as bacc
    import concourse.mybir as mybir
    import concourse.tile as tile
    from concourse.masks import make_identity

    f32 = mybir.dt.float32
    f16 = mybir.dt.float16
    AF = mybir.ActivationFunctionType

    nc = bacc.Bacc(None, target_bir_lowering=False)
    xq = nc.dram_tensor("xq", [CQ, D], f32, kind="ExternalInput")
    xk = nc.dram_tensor("xk", [C, D], f32, kind="ExternalInput")
    wcat = nc.dram_tensor("wcat", [C, 2 * D], f32, kind="ExternalInput")
    wg = nc.dram_tensor("wg", [CQ, D], f32, kind="ExternalInput")
    bg = nc.dram_tensor("bg", [CQ, D], f32, kind="ExternalInput")
    out = nc.dram_tensor("out", [CQ, D], f32, kind="ExternalOutput")

    with tile.TileContext(nc) as tc:
        with (
            tc.tile_pool(name="singles", bufs=1) as singles,
            tc.tile_pool(name="sb", bufs=2) as sb,
            tc.tile_pool(name="exp", bufs=4) as expp,
            tc.tile_pool(name="epair", bufs=3) as epp,
            tc.tile_pool(name="fin", bufs=4) as fin,
            tc.tile_pool(name="stage", bufs=3, space="PSUM") as stage,
            tc.tile_pool(name="acc", bufs=1, space="PSUM") as accp,
            tc.tile_pool(name="den", bufs=1, space="PSUM") as denp,
        ):
            # ---------------- loads ----------------
            # qbig first (stage-1 rhs is on the critical path), kbig and wcat
            # chunked so norms/casts/transposes pipeline with the tail of the
            # loads; wg/bg last (only needed at finalize).
            xk_r = xk.rearrange("(t p) d -> p t d", p=128)
            wc_r = wcat.rearrange("(t p) m -> p t m", p=128)
            qbig = singles.tile([128, QT, D], f32)
            nc.sync.dma_start(out=qbig, in_=xq.rearrange("(t p) d -> p t d", p=128))
            kbig = singles.tile([128, KT, D], f32)
            for c in range(4):
                nc.sync.dma_start(
                    out=kbig[:, 8 * c : 8 * (c + 1), :], in_=xk_r[:, 8 * c : 8 * (c + 1), :]
                )
            wsb = singles.tile([128, KT, 2 * D], f32)
            for c in range(4):
                nc.sync.dma_start(
                    out=wsb[:, 8 * c : 8 * (c + 1), :], in_=wc_r[:, 8 * c : 8 * (c + 1), :]
                )
            wgs = singles.tile([128, QT, D], f32)
            nc.sync.dma_start(out=wgs, in_=wg.rearrange("(t p) d -> p t d", p=128))
            bgs = singles.tile([128, QT, D], f32)
            nc.sync.dma_start(out=bgs, in_=bg.rearrange("(t p) d -> p t d", p=128))

            identity = singles.tile([128, 128], f32)
            make_identity(nc, identity)
            identity16 = singles.tile([128, 128], f16)
            nc.gpsimd.tensor_copy(out=identity16, in_=identity)
            ones16 = singles.tile([128, 1], f16)
            nc.vector.memset(ones16, 1.0)

            # ---------------- norms ----------------
            ktmp = sb.tile([128, KT, D], f32, tag="ktmp")
            ksq = singles.tile([128, KT], f32)
            for c in range(4):
                cs = slice(8 * c, 8 * (c + 1))
                nc.vector.tensor_mul(ktmp[:, cs, :], kbig[:, cs, :], kbig[:, cs, :])
                nc.vector.reduce_sum(
                    out=ksq[:, cs], in_=ktmp[:, cs, :], axis=mybir.AxisListType.X
                )
            # sqrt(0.01*s) = ||k||/10 ; reciprocal -> 10/||k||
            kscale = singles.tile([128, KT], f32)
            nc.scalar.activation(out=kscale, in_=ksq, func=AF.Sqrt, scale=0.01)
            nc.vector.reciprocal(out=kscale, in_=kscale)

            qtmp = sb.tile([128, QT, D], f32, tag="ktmp")
            nc.vector.tensor_mul(qtmp, qbig, qbig)
            qsq = singles.tile([128, QT], f32)
            nc.vector.reduce_sum(out=qsq, in_=qtmp, axis=mybir.AxisListType.X)
            qscale = singles.tile([128, QT], f32)
            nc.scalar.activation(out=qscale, in_=qsq, func=AF.Sqrt, scale=1.0)
            nc.vector.reciprocal(out=qscale, in_=qscale)

            # normalized fp16 copies (scale folded): kn = (10/||k||) * k, qn = q/||q||
            kn16 = singles.tile([128, KT, D], f16)
            for t in range(KT):
                nc.vector.tensor_scalar_mul(
                    out=kn16[:, t, :], in0=kbig[:, t, :], scalar1=kscale[:, t : t + 1]
                )
            qn16 = singles.tile([128, QT, D], f16)
            for t in range(QT):
                nc.vector.tensor_scalar_mul(
                    out=qn16[:, t, :], in0=qbig[:, t, :], scalar1=qscale[:, t : t + 1]
                )
            wsb16 = singles.tile([128, KT, 2 * D], f16)
            for c in range(4):
                nc.gpsimd.tensor_copy(
                    out=wsb16[:, 8 * c : 8 * (c + 1), :],
                    in_=wsb[:, 8 * c : 8 * (c + 1), :],
                )

            # winp = wg * x  (elementwise, per query row)
            winp = singles.tile([128, QT, D], f32)
            nc.vector.tensor_mul(winp, wgs, qbig)

            # ---------------- transposed layouts ----------------
            # PE-mode transposes (PE/ACT are idle during init, DMA queues are
            # not): [128, 64] tile -> psum [64, 128] -> ScalarE copy-cast to
            # fp16 SBUF. Everything lands on partitions 0-63 so stage-1 needs
            # no duplicated operands and no partition-moving fixups.
            qnT = singles.tile([64, QT, 128], f16)
            for t in range(QT):
                pt = stage.tile([64, 128], f16, tag="stage", name=f"ptq{t}")
                nc.tensor.transpose(pt, qn16[:, t, :], identity16)
                nc.scalar.copy(out=qnT[:, t, :], in_=pt)
            xkT = singles.tile([64, KT, 128], f16)
            for t in range(KT):
                pt = stage.tile([64, 128], f16, tag="stage", name=f"ptk{t}")
                nc.tensor.transpose(pt, kn16[:, t, :], identity16)
                if t % 2 == 0:
                    nc.scalar.copy(out=xkT[:, t, :], in_=pt)
                else:
                    nc.vector.tensor_copy(out=xkT[:, t, :], in_=pt)

            # ---------------- main loop ----------------
            # Quarter-sweeps: one 512-query chunk at a time. PSUM budget
            # (8 banks): stage 3x[128,2,512]=6, acc [128,512]=1, den [1,512]=1.
            # Software pipeline with 1-iteration skew; stage bufs=3 gives the
            # tensor engine lookahead so s1 overlaps the scalar-engine exps.
            out_nat = singles.tile([128, QT, D], f32)
            NJ = KT // 2  # 16 k-tile pairs
            for qc in range(4):
                acc_ps = accp.tile([128, 512], f32, tag="acc", name=f"acc{qc}")
                den_ps = denp.tile([1, 512], f32, tag="den", name=f"den{qc}")
                rhs = qnT[:, 4 * qc : 4 * qc + 4, :]

                exps = {}
                pairs = {}
                for j in range(NJ + 1):
                    if j < NJ:
                        st = stage.tile([128, 2, 512], f32, tag="stage",
                                        name=f"st{qc}_{j}")
                        nc.tensor.matmul(
                            st[:, 0, :], lhsT=xkT[:, 2 * j, :], rhs=rhs,
                            start=True, stop=True,
                        )
                        nc.tensor.matmul(
                            st[:, 1, :], lhsT=xkT[:, 2 * j + 1, :], rhs=rhs,
                            start=True, stop=True,
                        )
                        e = expp.tile([128, 2, 512], f16, tag="exp",
                                      name=f"e{qc}_{j}")
                        nc.scalar.activation(out=e, in_=st, func=AF.Exp)
                        # fp16 pair-sum for the denominator (2*e^10 < fp16 max)
                        ep = epp.tile([128, 512], f16, tag="epair",
                                      name=f"ep{qc}_{j}")
                        nc.vector.tensor_add(ep, e[:, 0, :], e[:, 1, :])
                        exps[j] = e
                        pairs[j] = ep
                    if j > 0:
                        jj = j - 1
                        e = exps[jj]
                        for par in range(2):  # k-tile 2*jj + par
                            kt = 2 * jj + par
                            nc.tensor.matmul(
                                acc_ps, lhsT=wsb16[:, kt, :], rhs=e[:, par, :],
                                start=(kt == 0), stop=(kt == KT - 1),
                                skip_group_check=True,
                            )
                        nc.tensor.matmul(
                            den_ps, lhsT=ones16, rhs=pairs[jj],
                            start=(jj == 0), stop=(jj == NJ - 1),
                            skip_group_check=True,
                        )

                # ---------------- finalize ----------------
                rinv = fin.tile([1, 512], f32, tag="rinv")
                nc.vector.reciprocal(out=rinv, in_=den_ps)
                rb = fin.tile([128, 512], f32, tag="rb")
                nc.gpsimd.partition_broadcast(rb, rinv)
                accs = fin.tile([128, 512], f32, tag="accs")
                nc.vector.tensor_mul(accs, acc_ps, rb)
                for sub in range(4):
                    qt = qc * 4 + sub
                    ot = stage.tile([128, 2, 512], f32, tag="stage",
                                    name=f"ot{qc}_{sub}")
                    nc.tensor.transpose(
                        ot[:, 0, 0:128],
                        accs[:, sub * 128 : (sub + 1) * 128],
                        identity,
                    )
                    t1 = fin.tile([128, D], f32, tag="t1", name=f"t1_{qc}_{sub}")
                    nc.vector.tensor_mul(t1, ot[:, 0, 0:64], winp[:, qt, :])
                    t2 = fin.tile([128, D], f32, tag="t2", name=f"t2_{qc}_{sub}")
                    nc.vector.tensor_mul(t2, ot[:, 0, 64:128], bgs[:, qt, :])
                    nc.vector.tensor_add(out_nat[:, qt, :], t1, t2)

            nc.sync.dma_start(
                out=out.rearrange("(t p) d -> p t d", p=128), in_=out_nat
            )

    nc.compile()
    return nc


def _get_nc():
    if "nc" not in _CACHE:
        _CACHE["nc"] = _build()
    return _CACHE["nc"]


def _make_in_maps(input, weight, bias, weight_global, bias_global):
    input = np.ascontiguousarray(np.asarray(input, dtype=np.float32))
    ones = lambda: np.ones((C, D), np.float32)
    weight = ones() if weight is None else np.asarray(weight, np.float32)
    bias = np.zeros((C, D), np.float32) if bias is None else np.asarray(bias, np.float32)
    weight_global = ones() if weight_global is None else np.asarray(weight_global, np.float32)
    bias_global = ones() if bias_global is None else np.asarray(bias_global, np.float32)
    wcat = np.ascontiguousarray(np.concatenate([weight, bias], axis=1))
    in_maps = []
    for core in range(NCORES):
        b, h = divmod(core, 2)
        sl = slice(h * CQ, (h + 1) * CQ)
        in_maps.append({
            "xq": np.ascontiguousarray(input[b, sl]),
            "xk": np.ascontiguousarray(input[b]),
            "wcat": wcat,
            "wg": np.ascontiguousarray(weight_global[sl]),
            "bg": np.ascontiguousarray(bias_global[sl]),
        })
    return in_maps


def _run(in_maps, **kw):
    from concourse.bass_utils import run_bass_kernel_spmd
    nc = _get_nc()
    return run_bass_kernel_spmd(nc, in_maps, core_ids=list(range(NCORES)), **kw)


def kernel(input, weight=None, bias=None, weight_global=None, bias_global=None,
           **_ignored):
    in_maps = _make_in_maps(input, weight, bias, weight_global, bias_global)
    res = _run(in_maps)
    out = np.empty((B, C, D), np.float32)
    for core in range(NCORES):
        b, h = divmod(core, 2)
        out[b, h * CQ : (h + 1) * CQ] = res.results[core]["out"]
    return out



# revision 4
# speedup vs baseline: 1.0766x; 1.0766x over previous
"""Trainium2 Bass kernel for CosineSimilarityWeightedAverage.

reference:
  input [B=4, C=4096, D=64] f32
  in_n = input / ||input||_row
  cos  = in_n @ in_n.T per batch            [B, C, C]
  attn = softmax(cos / 0.1, axis=-1)
  out  = (attn @ weight) * weight_global * input + (attn @ bias) * bias_global

Sharding: 8 cores = (batch b = core//2) x (query half h = core%2, 2048 rows).
Each core sees all 4096 keys of its batch (permuted so its own queries come
first) and computes 2048 output rows.

Per-core dataflow (v2):
  - host supplies layout-only transforms: f16 cast of x, f16 transposed keys
    xkT [64, 4096] (so no on-device key transposes), and [W|bias] pre-packed
    in fp8e5 DoubleRow layout [128, 16, 2, 128].
  - keys stay UNNORMALIZED; the 10/||k|| factor (temperature folded) is a
    per-partition scalar applied inside the exp (activation scale AP / DVE
    tensor_scalar scalar AP) in the [k, q] score layout.
  - queries are normalized on device (16 tiles) and PE-transposed.
  - stage 1: st[k, q] = xkT.T @ qnT per k-tile (f16, K=64).
  - exp split across engines: ACT runs native Exp -> fp8e5; DVE fabricates
    the e5m2 bits with the exp2 bit trick (i8 = 4*log2e*scale*st + 60.5,
    truncated, bitcast to fp8e5). Both cancel exactly in softmax.
  - stage 2 + denominator: fp8e5 DoubleRow matmuls (2 k-tiles per matmul,
    0.5 cycles/row): attn-num [128cols, 512q] and den [1, 512q] accumulate
    in PSUM f32 over the 16 k-tile pairs.
  - finalize per 512-query chunk: reciprocal, partition-broadcast, normalize,
    PE transpose back to [q, d], out = avgW*(wg*x) + avgB*bg, DMA out.
"""

import numpy as np

B = 4
C = 4096
D = 64
NCORES = 8
CQ = C // 2          # queries per core
KT = C // 128        # 32 k-tiles
QT = CQ // 128       # 16 q-tiles
NJ = KT // 2         # 16 k-tile pairs
LOG2E = 1.4426950408889634

# exp routing: k-tile pairs handled by the DVE bit-trick (rest go to ACT)
DVE_PAIRS = frozenset({2, 4, 7, 9, 12, 14})

_CACHE = {}


def _build():
    import concourse.bass as bass
    import concourse.bacc as bacc
    import concourse.mybir as mybir
    import concourse.tile as tile
    from concourse.masks import make_identity

    f32 = mybir.dt.float32
    f16 = mybir.dt.float16
    f8 = mybir.dt.float8e5
    i8 = mybir.dt.int8
    AF = mybir.ActivationFunctionType
    DR = mybir.MatmulPerfMode.DoubleRow
    ALU = mybir.AluOpType

    nc = bacc.Bacc(None, target_bir_lowering=False)
    xq16 = nc.dram_tensor("xq16", [CQ, D], f16, kind="ExternalInput")
    xk16 = nc.dram_tensor("xk16", [C, D], f16, kind="ExternalInput")
    xkT = nc.dram_tensor("xkT", [D, C], f16, kind="ExternalInput")
    wsb8 = nc.dram_tensor("wsb8", [128, NJ, 2, 2 * D], f8, kind="ExternalInput")
    wg = nc.dram_tensor("wg", [CQ, D], f32, kind="ExternalInput")
    bg = nc.dram_tensor("bg", [CQ, D], f32, kind="ExternalInput")
    out = nc.dram_tensor("out", [CQ, D], f32, kind="ExternalOutput")

    with tile.TileContext(nc) as tc:
        with (
            tc.tile_pool(name="singles", bufs=1) as singles,
            tc.tile_pool(name="sb", bufs=2) as sb,
            tc.tile_pool(name="exp", bufs=6) as expp,
            tc.tile_pool(name="fin", bufs=2) as fin,
            tc.tile_pool(name="stage", bufs=2, space="PSUM") as stage,
            tc.tile_pool(name="acc", bufs=2, space="PSUM") as accp,
            tc.tile_pool(name="den", bufs=1, space="PSUM") as denp,
        ):
            # ---------------- loads ----------------
            # critical path first: xq16 (q norms + transposes), xk16 chunks
            # (k norms feed the exp scales), xkT chunks (stage-1 lhsT).
            # xkT + wsb8 go on the Pool DGE queue so they stream in parallel
            # with the sync-queue loads.
            xq_s = singles.tile([128, QT, D], f16)
            nc.sync.dma_start(out=xq_s, in_=xq16.rearrange("(t p) d -> p t d", p=128))
            xk_s = singles.tile([128, KT, D], f16)
            xk_r = xk16.rearrange("(t p) d -> p t d", p=128)
            for c in range(4):
                nc.sync.dma_start(
                    out=xk_s[:, 8 * c : 8 * (c + 1), :],
                    in_=xk_r[:, 8 * c : 8 * (c + 1), :],
                )
            xkT_s = singles.tile([64, KT, 128], f16)
            xkT_r = xkT.rearrange("d (t k) -> d t k", k=128)
            for c in range(4):
                nc.gpsimd.dma_start(
                    out=xkT_s[:, 8 * c : 8 * (c + 1), :],
                    in_=xkT_r[:, 8 * c : 8 * (c + 1), :],
                )
            wsb_s = singles.tile([128, NJ, 2, 2 * D], f8)
            nc.gpsimd.dma_start(out=wsb_s, in_=wsb8[:, :, :, :])
            wgs = singles.tile([128, QT, D], f32)
            nc.sync.dma_start(out=wgs, in_=wg.rearrange("(t p) d -> p t d", p=128))
            bgs = singles.tile([128, QT, D], f32)
            nc.sync.dma_start(out=bgs, in_=bg.rearrange("(t p) d -> p t d", p=128))

            identity = singles.tile([128, 128], f32)
            make_identity(nc, identity)
            identity16 = singles.tile([128, 128], f16)
            nc.gpsimd.tensor_copy(out=identity16, in_=identity)
            ones8 = singles.tile([128, 2, 32], f8)
            nc.vector.memset(ones8, 1.0)

            # ---------------- norms ----------------
            # q: qscale = 1/||q||; k: kinv10 = 10/||k|| (temperature folded),
            # kdve = kinv10 * 4*log2e for the DVE exp bit-trick.
            qsq = singles.tile([128, QT], f32)
            for c in range(2):
                cs = slice(8 * c, 8 * (c + 1))
                qtmp = sb.tile([128, 8, D], f32, tag="sqt", name=f"qtmp{c}")
                nc.vector.tensor_mul(qtmp, xq_s[:, cs, :], xq_s[:, cs, :])
                nc.vector.reduce_sum(out=qsq[:, cs], in_=qtmp, axis=mybir.AxisListType.X)
            qscale = singles.tile([128, QT], f32)
            nc.scalar.activation(out=qscale, in_=qsq, func=AF.Sqrt, scale=1.0)
            nc.vector.reciprocal(out=qscale, in_=qscale)

            ksq = singles.tile([128, KT], f32)
            kinv10 = singles.tile([128, KT], f32)
            kdve = singles.tile([128, KT], f32)
            for c in range(4):
                cs = slice(8 * c, 8 * (c + 1))
                ktmp = sb.tile([128, 8, D], f32, tag="sqt", name=f"ktmp{c}")
                nc.vector.tensor_mul(ktmp, xk_s[:, cs, :], xk_s[:, cs, :])
                nc.vector.reduce_sum(out=ksq[:, cs], in_=ktmp, axis=mybir.AxisListType.X)
                # sqrt(0.01*s) = ||k||/10 ; reciprocal -> 10/||k||
                nc.scalar.activation(
                    out=kinv10[:, cs], in_=ksq[:, cs], func=AF.Sqrt, scale=0.01
                )
                nc.vector.reciprocal(out=kinv10[:, cs], in_=kinv10[:, cs])
                nc.vector.tensor_scalar_mul(
                    out=kdve[:, cs], in0=kinv10[:, cs], scalar1=4.0 * LOG2E
                )

            # winp = wg * x  (elementwise, per query row)
            winp = singles.tile([128, QT, D], f32)
            nc.vector.tensor_mul(winp, wgs, xq_s)

            # normalized queries (f16) + PE transpose to [64, q]
            qn16 = singles.tile([128, QT, D], f16)
            for t in range(QT):
                nc.vector.tensor_scalar_mul(
                    out=qn16[:, t, :], in0=xq_s[:, t, :], scalar1=qscale[:, t : t + 1]
                )
            qnT = singles.tile([64, QT, 128], f16)
            for bk in range(2):
                pt = stage.tile([64, 8, 128], f16, tag="stage", name=f"ptq{bk}")
                for s in range(8):
                    t = 8 * bk + s
                    nc.tensor.transpose(pt[:, s, :], qn16[:, t, :], identity16)
                nc.vector.tensor_copy(out=qnT[:, 8 * bk : 8 * (bk + 1), :], in_=pt)

            # ---------------- main loop ----------------
            # Quarter-sweeps: one 512-query chunk at a time. PSUM budget
            # (8 banks): stage 2x[128,2,512]=4 + ot 1, acc 2x[128,512]=2,
            # den 1. Software pipeline with 1-iteration skew.
            for qc in range(4):
                acc_ps = accp.tile([128, 512], f32, tag="acc", name=f"acc{qc}")
                den_ps = denp.tile([32, 512], f32, tag="den", name=f"den{qc}")
                rhs = qnT[:, 4 * qc : 4 * qc + 4, :]

                exps = {}
                for j in range(NJ + 1):
                    if j < NJ:
                        st = stage.tile([128, 2, 512], f32, tag="stage",
                                        name=f"st{qc}_{j}")
                        nc.tensor.matmul(
                            st[:, 0, :], lhsT=xkT_s[:, 2 * j, :], rhs=rhs,
                            start=True, stop=True,
                        )
                        nc.tensor.matmul(
                            st[:, 1, :], lhsT=xkT_s[:, 2 * j + 1, :], rhs=rhs,
                            start=True, stop=True,
                        )
                        e8 = expp.tile([128, 2, 512], f8, tag="exp",
                                       name=f"e{qc}_{j}")
                        if j in DVE_PAIRS:
                            e8i = e8.bitcast(i8)
                            for par in range(2):
                                kt = 2 * j + par
                                nc.vector.tensor_scalar(
                                    out=e8i[:, par, :], in0=st[:, par, :],
                                    scalar1=kdve[:, kt : kt + 1], scalar2=60.5,
                                    op0=ALU.mult, op1=ALU.add,
                                )
                        else:
                            for par in range(2):
                                kt = 2 * j + par
                                nc.scalar.activation(
                                    out=e8[:, par, :], in_=st[:, par, :],
                                    func=AF.Exp, scale=kinv10[:, kt : kt + 1],
                                )
                        exps[j] = e8
                    if j > 0:
                        jj = j - 1
                        e = exps.pop(jj)
                        nc.tensor.matmul(
                            acc_ps, lhsT=wsb_s[:, jj], rhs=e, perf_mode=DR,
                            start=(jj == 0), stop=(jj == NJ - 1),
                            skip_group_check=True,
                        )
                        nc.tensor.matmul(
                            den_ps, lhsT=ones8, rhs=e, perf_mode=DR,
                            start=(jj == 0), stop=(jj == NJ - 1),
                            skip_group_check=True,
                        )

                # ---------------- finalize ----------------
                rinv = fin.tile([1, 512], f32, tag="rinv")
                nc.vector.reciprocal(out=rinv, in_=den_ps[0:1, :])
                rb = fin.tile([128, 512], f32, tag="rb")
                nc.gpsimd.partition_broadcast(rb, rinv)
                accs = fin.tile([128, 512], f32, tag="accs")
                nc.vector.tensor_mul(accs, acc_ps, rb)
                ot = stage.tile([128, 512], f32, tag="stage", name=f"ot{qc}")
                for s in range(4):
                    nc.tensor.transpose(
                        ot[:, s * 128 : (s + 1) * 128],
                        accs[:, s * 128 : (s + 1) * 128],
                        identity,
                    )
                ot4 = ot.rearrange("p (s k) -> p s k", s=4)
                qs = slice(4 * qc, 4 * qc + 4)
                t1 = fin.tile([128, 4, D], f32, tag="t1")
                nc.vector.tensor_mul(t1, ot4[:, :, 0:64], winp[:, qs, :])
                t2 = fin.tile([128, 4, D], f32, tag="t2")
                nc.vector.tensor_mul(t2, ot4[:, :, 64:128], bgs[:, qs, :])
                onat = fin.tile([128, 4, D], f32, tag="onat")
                nc.vector.tensor_add(onat, t1, t2)
                nc.sync.dma_start(
                    out=out.rearrange("(t p) d -> p t d", p=128)[:, qs, :],
                    in_=onat,
                )

    nc.compile()
    return nc


def _get_nc():
    if "nc" not in _CACHE:
        _CACHE["nc"] = _build()
    return _CACHE["nc"]


def _make_in_maps(input, weight, bias, weight_global, bias_global):
    import ml_dtypes

    f8 = ml_dtypes.float8_e5m2
    input = np.ascontiguousarray(np.asarray(input, dtype=np.float32))
    ones = lambda: np.ones((C, D), np.float32)
    weight = ones() if weight is None else np.asarray(weight, np.float32)
    bias = np.zeros((C, D), np.float32) if bias is None else np.asarray(bias, np.float32)
    weight_global = ones() if weight_global is None else np.asarray(weight_global, np.float32)
    bias_global = ones() if bias_global is None else np.asarray(bias_global, np.float32)
    wcat = np.concatenate([weight, bias], axis=1)  # [C, 128]

    # per-half key permutation (own queries first) + DoubleRow fp8 layout:
    # wsb8[p, j, h, c] = wcat_perm[(2j+h)*128 + p, c]
    def dr_pack(wc):
        return np.ascontiguousarray(
            wc.reshape(NJ, 2, 128, 2 * D).transpose(2, 0, 1, 3).astype(f8)
        )

    wsb8_h = [
        dr_pack(wcat),
        dr_pack(np.concatenate([wcat[CQ:], wcat[:CQ]], axis=0)),
    ]

    in_maps = []
    for core in range(NCORES):
        b, h = divmod(core, 2)
        sl = slice(h * CQ, (h + 1) * CQ)
        xb = input[b]
        xperm = xb if h == 0 else np.concatenate([xb[CQ:], xb[:CQ]], axis=0)
        xk16 = np.ascontiguousarray(xperm.astype(np.float16))
        in_maps.append({
            "xq16": xk16[:CQ].copy(),
            "xk16": xk16,
            "xkT": np.ascontiguousarray(xk16.T),
            "wsb8": wsb8_h[h],
            "wg": np.ascontiguousarray(weight_global[sl]),
            "bg": np.ascontiguousarray(bias_global[sl]),
        })
    return in_maps


def _run(in_maps, **kw):
    from concourse.bass_utils import run_bass_kernel_spmd
    nc = _get_nc()
    return run_bass_kernel_spmd(nc, in_maps, core_ids=list(range(NCORES)), **kw)


def kernel(input, weight=None, bias=None, weight_global=None, bias_global=None,
           **_ignored):
    in_maps = _make_in_maps(input, weight, bias, weight_global, bias_global)
    res = _run(in_maps)
    out = np.empty((B, C, D), np.float32)
    for core in range(NCORES):
        b, h = divmod(core, 2)
        out[b, h * CQ : (h + 1) * CQ] = res.results[core]["out"]
    return out


# revision 8
# speedup vs baseline: 1.5765x; 1.4643x over previous
"""Trainium2 Bass kernel for CosineSimilarityWeightedAverage.

reference:
  input [B=4, C=4096, D=64] f32
  in_n = input / ||input||_row
  cos  = in_n @ in_n.T per batch            [B, C, C]
  attn = softmax(cos / 0.1, axis=-1)
  out  = (attn @ weight) * weight_global * input + (attn @ bias) * bias_global

Sharding: 8 cores = (batch b = core//2) x (query half h = core%2, 2048 rows).
Each core sees all 4096 keys of its batch (permuted so its own queries come
first) and computes 2048 output rows.

Per-core dataflow (v2):
  - host supplies layout-only transforms: f16 cast of x, f16 transposed keys
    xkT [64, 4096] (so no on-device key transposes), and [W|bias] pre-packed
    in fp8e5 DoubleRow layout [128, 16, 2, 128].
  - keys stay UNNORMALIZED; the 10/||k|| factor (temperature folded) is a
    per-partition scalar applied inside the exp (activation scale AP / DVE
    tensor_scalar scalar AP) in the [k, q] score layout.
  - queries are normalized on device (16 tiles) and PE-transposed.
  - stage 1: st[k, q] = xkT.T @ qnT per k-tile (f16, K=64).
  - exp split across engines: ACT runs native Exp -> fp8e5; DVE fabricates
    the e5m2 bits with the exp2 bit trick (i8 = 4*log2e*scale*st + 60.5,
    truncated, bitcast to fp8e5). Both cancel exactly in softmax.
  - stage 2 + denominator: fp8e5 DoubleRow matmuls (2 k-tiles per matmul,
    0.5 cycles/row): attn-num [128cols, 512q] and den [1, 512q] accumulate
    in PSUM f32 over the 16 k-tile pairs.
  - finalize per 512-query chunk: reciprocal, partition-broadcast, normalize,
    PE transpose back to [q, d], out = avgW*(wg*x) + avgB*bg, DMA out.
"""

import numpy as np

B = 4
C = 4096
D = 64
NCORES = 8
CQ = C // 2          # queries per core
KT = C // 128        # 32 k-tiles
QT = CQ // 128       # 16 q-tiles
NJ = KT // 2         # 16 k-tile pairs
LOG2E = 1.4426950408889634

# exp routing: k-tile pairs handled by the DVE bit-trick (rest go to ACT).
# Strict alternation so consecutive pairs overlap on different engines.
DVE_PAIRS = frozenset({1, 3, 5, 7, 9, 11, 13})

_CACHE = {}


def _build():
    import concourse.bass as bass
    import concourse.bacc as bacc
    import concourse.mybir as mybir
    import concourse.tile as tile
    from concourse.masks import make_identity

    f32 = mybir.dt.float32
    f16 = mybir.dt.float16
    f8 = mybir.dt.float8e5
    i8 = mybir.dt.int8
    AF = mybir.ActivationFunctionType
    DR = mybir.MatmulPerfMode.DoubleRow
    ALU = mybir.AluOpType

    nc = bacc.Bacc(None, target_bir_lowering=False)
    xq16 = nc.dram_tensor("xq16", [CQ, D], f16, kind="ExternalInput")
    xk16 = nc.dram_tensor("xk16", [C, D], f16, kind="ExternalInput")
    xkT = nc.dram_tensor("xkT", [D, C], f16, kind="ExternalInput")
    wsb8 = nc.dram_tensor("wsb8", [128, NJ, 2, 2 * D], f8, kind="ExternalInput")
    wg = nc.dram_tensor("wg", [CQ, D], f32, kind="ExternalInput")
    bg = nc.dram_tensor("bg", [CQ, D], f32, kind="ExternalInput")
    out = nc.dram_tensor("out", [CQ, D], f32, kind="ExternalOutput")

    with tile.TileContext(nc) as tc:
        with (
            tc.tile_pool(name="singles", bufs=1) as singles,
            tc.tile_pool(name="sb", bufs=2) as sb,
            tc.tile_pool(name="exp", bufs=6) as expp,
            tc.tile_pool(name="fin", bufs=2) as fin,
            tc.tile_pool(name="stage", bufs=4, space="PSUM") as stage,
            tc.tile_pool(name="otp", bufs=1, space="PSUM") as otp,
            tc.tile_pool(name="acc", bufs=2, space="PSUM") as accp,
            tc.tile_pool(name="den", bufs=1, space="PSUM") as denp,
        ):
            # ---------------- loads ----------------
            # critical path first: xq16 (q norms + transposes), xk16 chunks
            # (k norms feed the exp scales), xkT chunks (stage-1 lhsT).
            # xkT + wsb8 go on the Pool DGE queue so they stream in parallel
            # with the sync-queue loads.
            xq_s = singles.tile([128, QT, D], f16)
            nc.sync.dma_start(out=xq_s, in_=xq16.rearrange("(t p) d -> p t d", p=128))
            xk_s = singles.tile([128, KT, D], f16)
            xk_r = xk16.rearrange("(t p) d -> p t d", p=128)
            for c in range(4):
                nc.sync.dma_start(
                    out=xk_s[:, 8 * c : 8 * (c + 1), :],
                    in_=xk_r[:, 8 * c : 8 * (c + 1), :],
                )
            xkT_s = singles.tile([64, KT, 128], f16)
            xkT_r = xkT.rearrange("d (t k) -> d t k", k=128)
            for c in range(4):
                nc.gpsimd.dma_start(
                    out=xkT_s[:, 8 * c : 8 * (c + 1), :],
                    in_=xkT_r[:, 8 * c : 8 * (c + 1), :],
                )
            wsb_s = singles.tile([128, NJ, 2, 2 * D], f8)
            nc.gpsimd.dma_start(out=wsb_s, in_=wsb8[:, :, :, :])
            wgs = singles.tile([128, QT, D], f32)
            nc.sync.dma_start(out=wgs, in_=wg.rearrange("(t p) d -> p t d", p=128))
            bgs = singles.tile([128, QT, D], f32)
            nc.sync.dma_start(out=bgs, in_=bg.rearrange("(t p) d -> p t d", p=128))

            identity = singles.tile([128, 128], f32)
            make_identity(nc, identity)
            identity16 = singles.tile([128, 128], f16)
            nc.gpsimd.tensor_copy(out=identity16, in_=identity)
            ones8 = singles.tile([128, 2, 32], f8)
            nc.vector.memset(ones8, 1.0)

            # ---------------- norms ----------------
            # q: qscale = 1/||q||; k: kinv10 = 10/||k|| (temperature folded),
            # kdve = kinv10 * 4*log2e for the DVE exp bit-trick.
            qsq = singles.tile([128, QT], f32)
            for c in range(2):
                cs = slice(8 * c, 8 * (c + 1))
                qtmp = sb.tile([128, 8, D], f32, tag="sqt", name=f"qtmp{c}")
                nc.vector.tensor_mul(qtmp, xq_s[:, cs, :], xq_s[:, cs, :])
                nc.vector.reduce_sum(out=qsq[:, cs], in_=qtmp, axis=mybir.AxisListType.X)
            qscale = singles.tile([128, QT], f32)
            nc.scalar.activation(out=qscale, in_=qsq, func=AF.Sqrt, scale=1.0)
            nc.vector.reciprocal(out=qscale, in_=qscale)

            ksq = singles.tile([128, KT], f32)
            kinv10 = singles.tile([128, KT], f32)
            kdve = singles.tile([128, KT], f32)
            for c in range(4):
                cs = slice(8 * c, 8 * (c + 1))
                ktmp = sb.tile([128, 8, D], f32, tag="sqt", name=f"ktmp{c}")
                nc.vector.tensor_mul(ktmp, xk_s[:, cs, :], xk_s[:, cs, :])
                nc.vector.reduce_sum(out=ksq[:, cs], in_=ktmp, axis=mybir.AxisListType.X)
                # sqrt(0.01*s) = ||k||/10 ; reciprocal -> 10/||k||
                nc.scalar.activation(
                    out=kinv10[:, cs], in_=ksq[:, cs], func=AF.Sqrt, scale=0.01
                )
                nc.vector.reciprocal(out=kinv10[:, cs], in_=kinv10[:, cs])
                nc.vector.tensor_scalar_mul(
                    out=kdve[:, cs], in0=kinv10[:, cs], scalar1=4.0 * LOG2E
                )

            # winp = wg * x  (elementwise, per query row)
            winp = singles.tile([128, QT, D], f32)
            nc.vector.tensor_mul(winp, wgs, xq_s)

            # normalized queries (f16) + PE transpose to [64, q]
            qn16 = singles.tile([128, QT, D], f16)
            for t in range(QT):
                nc.vector.tensor_scalar_mul(
                    out=qn16[:, t, :], in0=xq_s[:, t, :], scalar1=qscale[:, t : t + 1]
                )
            qnT = singles.tile([64, QT, 128], f16)
            for bk in range(2):
                pt = stage.tile([64, 8, 128], f16, tag="stage", name=f"ptq{bk}")
                for s in range(8):
                    t = 8 * bk + s
                    nc.tensor.transpose(pt[:, s, :], qn16[:, t, :], identity16)
                nc.vector.tensor_copy(out=qnT[:, 8 * bk : 8 * (bk + 1), :], in_=pt)

            # ---------------- main loop ----------------
            # Quarter-sweeps: one 512-query chunk at a time. PSUM budget
            # (8 banks): stage 4x[128,512]=4, ot 1, acc 2x[128,512]=2,
            # den 1. Software pipeline with 1-iteration skew; half-pair
            # st tiles keep 2 pairs in flight so the alternating ACT/DVE
            # exps overlap.
            for qc in range(4):
                acc_ps = accp.tile([128, 512], f32, tag="acc", name=f"acc{qc}")
                den_ps = denp.tile([32, 512], f32, tag="den", name=f"den{qc}")
                rhs = qnT[:, 4 * qc : 4 * qc + 4, :]

                exps = {}
                for j in range(NJ + 1):
                    if j < NJ:
                        e8 = expp.tile([128, 2, 512], f8, tag="exp",
                                       name=f"e{qc}_{j}")
                        e8i = e8.bitcast(i8)
                        for par in range(2):
                            kt = 2 * j + par
                            st = stage.tile([128, 512], f32, tag="stage",
                                            name=f"st{qc}_{kt}")
                            nc.tensor.matmul(
                                st, lhsT=xkT_s[:, kt, :], rhs=rhs,
                                start=True, stop=True,
                            )
                            if j in DVE_PAIRS:
                                nc.vector.tensor_scalar(
                                    out=e8i[:, par, :], in0=st,
                                    scalar1=kdve[:, kt : kt + 1], scalar2=60.5,
                                    op0=ALU.mult, op1=ALU.add,
                                )
                            else:
                                nc.scalar.activation(
                                    out=e8[:, par, :], in_=st,
                                    func=AF.Exp, scale=kinv10[:, kt : kt + 1],
                                )
                        exps[j] = e8
                    if j > 0:
                        jj = j - 1
                        e = exps.pop(jj)
                        nc.tensor.matmul(
                            acc_ps, lhsT=wsb_s[:, jj], rhs=e, perf_mode=DR,
                            start=(jj == 0), stop=(jj == NJ - 1),
                            skip_group_check=True,
                        )
                        nc.tensor.matmul(
                            den_ps, lhsT=ones8, rhs=e, perf_mode=DR,
                            start=(jj == 0), stop=(jj == NJ - 1),
                            skip_group_check=True,
                        )

                # ---------------- finalize ----------------
                # reciprocal on DVE (fires right after the last exp, freeing
                # den for the next chunk); broadcast + elementwise on Pool so
                # DVE/ACT stay on exp work.
                rinv = fin.tile([1, 512], f32, tag="rinv")
                nc.vector.reciprocal(out=rinv, in_=den_ps[0:1, :])
                rb = fin.tile([128, 512], f32, tag="rb")
                nc.gpsimd.partition_broadcast(rb, rinv)
                accs = fin.tile([128, 512], f32, tag="accs")
                nc.vector.tensor_mul(accs, acc_ps, rb)
                ot = otp.tile([128, 512], f32, tag="ot", name=f"ot{qc}")
                for s in range(4):
                    nc.tensor.transpose(
                        ot[:, s * 128 : (s + 1) * 128],
                        accs[:, s * 128 : (s + 1) * 128],
                        identity,
                    )
                ots = fin.tile([128, 512], f32, tag="ots")
                nc.scalar.copy(out=ots, in_=ot)
                ot4 = ots.rearrange("p (s k) -> p s k", s=4)
                qs = slice(4 * qc, 4 * qc + 4)
                t1 = fin.tile([128, 4, D], f32, tag="t1")
                nc.gpsimd.tensor_mul(t1, ot4[:, :, 0:64], winp[:, qs, :])
                t2 = fin.tile([128, 4, D], f32, tag="t2")
                nc.gpsimd.tensor_mul(t2, ot4[:, :, 64:128], bgs[:, qs, :])
                onat = fin.tile([128, 4, D], f32, tag="onat")
                nc.gpsimd.tensor_add(onat, t1, t2)
                nc.sync.dma_start(
                    out=out.rearrange("(t p) d -> p t d", p=128)[:, qs, :],
                    in_=onat,
                )

    nc.compile()
    return nc


def _get_nc():
    if "nc" not in _CACHE:
        _CACHE["nc"] = _build()
    return _CACHE["nc"]


def _make_in_maps(input, weight, bias, weight_global, bias_global):
    import ml_dtypes

    f8 = ml_dtypes.float8_e5m2
    input = np.ascontiguousarray(np.asarray(input, dtype=np.float32))
    ones = lambda: np.ones((C, D), np.float32)
    weight = ones() if weight is None else np.asarray(weight, np.float32)
    bias = np.zeros((C, D), np.float32) if bias is None else np.asarray(bias, np.float32)
    weight_global = ones() if weight_global is None else np.asarray(weight_global, np.float32)
    bias_global = ones() if bias_global is None else np.asarray(bias_global, np.float32)
    wcat = np.concatenate([weight, bias], axis=1)  # [C, 128]

    # per-half key permutation (own queries first) + DoubleRow fp8 layout:
    # wsb8[p, j, h, c] = wcat_perm[(2j+h)*128 + p, c]
    def dr_pack(wc):
        return np.ascontiguousarray(
            wc.reshape(NJ, 2, 128, 2 * D).transpose(2, 0, 1, 3).astype(f8)
        )

    wsb8_h = [
        dr_pack(wcat),
        dr_pack(np.concatenate([wcat[CQ:], wcat[:CQ]], axis=0)),
    ]

    in_maps = []
    for core in range(NCORES):
        b, h = divmod(core, 2)
        sl = slice(h * CQ, (h + 1) * CQ)
        xb = input[b]
        xperm = xb if h == 0 else np.concatenate([xb[CQ:], xb[:CQ]], axis=0)
        xk16 = np.ascontiguousarray(xperm.astype(np.float16))
        in_maps.append({
            "xq16": xk16[:CQ].copy(),
            "xk16": xk16,
            "xkT": np.ascontiguousarray(xk16.T),
            "wsb8": wsb8_h[h],
            "wg": np.ascontiguousarray(weight_global[sl]),
            "bg": np.ascontiguousarray(bias_global[sl]),
        })
    return in_maps


def _run(in_maps, **kw):
    from concourse.bass_utils import run_bass_kernel_spmd
    nc = _get_nc()
    return run_bass_kernel_spmd(nc, in_maps, core_ids=list(range(NCORES)), **kw)


def kernel(input, weight=None, bias=None, weight_global=None, bias_global=None,
           **_ignored):
    in_maps = _make_in_maps(input, weight, bias, weight_global, bias_global)
    res = _run(in_maps)
    out = np.empty((B, C, D), np.float32)
    for core in range(NCORES):
        b, h = divmod(core, 2)
        out[b, h * CQ : (h + 1) * CQ] = res.results[core]["out"]
    return out
